# revision 15
# baseline (speedup 1.0000x reference)
"""Grouped-GEMM (MoE routing) kernel for TRN2, 8 NeuronCores, SPMD.

out[m] = values[m] @ combining_matrix[species_idx[m]]
  values [131072, 128] f32, species_idx [131072] i32, combining_matrix [8, 128, 256] f32

Strategy:
  - Host: counting-sort rows by species; deal each species' rows round-robin
    across the 8 cores so per-core per-species counts are balanced (+-1).
    Each core's rows are packed species-contiguous into a transposed buffer
    xT [128, R_pad] (species segment s zero-padded to a static capacity C[s],
    identical on every core -> one SPMD program). All device traffic is bf16
    (host casts f32<->bf16 for free): ~13 MB/core HBM traffic vs ~27 MB for
    f32; f32 PSUM accumulate keeps scale-relative error at ~3e-3 (gate 2e-2).
  - Device (per core): all 8 weight matrices resident in SBUF (one DMA).
    Per species s, output half h: out_T[h*128+.., seg_s] =
    W[s][:, h*128:+128].T @ xT[:, seg_s] via 512-col matmul chunks (bf16 in,
    f32 PSUM). The PSUM->SBUF drain (with bf16 downcast) is the throughput-
    critical non-DMA stage: PSUM-src copies run at 1x mode (~1 elem/cycle/
    lane), so the two halves are drained by DIFFERENT engines in parallel —
    h=0 on DVE (tensor_copy), h=1 on ACT (activation copy). Out-DMAs are
    issued by the engine that produced the data (SP ring for h=0, ACT ring
    for h=1) so each engine only ever waits on its own prior work.
  - Host: scatter outT columns back to the full [131072, 256] f32 output.
"""

import numpy as np
import ml_dtypes
from contextlib import ExitStack

import concourse.bass as bass
import concourse.mybir as mybir
import concourse.tile as tile
from concourse import bacc
from concourse.bass_utils import run_bass_kernel_spmd

M_TOTAL = 131072
D_IN = 128
N_OUT = 256
N_SPECIES = 8
N_CORES = 8
PAD = 16           # species segment capacity granularity (rows)
CHUNK = 512        # matmul moving-dim chunk (PSUM bank limit)
GROUP = 1024       # PSUM tile cols (2 banks) drained by one cast
F32 = mybir.dt.float32
BF16 = ml_dtypes.bfloat16
MM_DT = mybir.dt.bfloat16
OUT_DT = mybir.dt.bfloat16


def _build_nc(caps, r_pad):
    """Build the SPMD program for one core. caps[s] = padded column count of
    species segment s (same on all cores); r_pad = sum(caps)."""
    nc = bacc.Bacc("TRN2", target_bir_lowering=False, debug=False,
                   num_devices=N_CORES)
    xT = nc.dram_tensor("xT", [D_IN, r_pad], MM_DT, kind="ExternalInput").ap()
    w = nc.dram_tensor("w", [D_IN, N_SPECIES * N_OUT], MM_DT,
                       kind="ExternalInput").ap()
    outT = nc.dram_tensor("outT", [N_OUT, r_pad], OUT_DT, kind="ExternalOutput").ap()

    max_cap = max(caps)
    offs = [0]
    for s in range(N_SPECIES):
        offs.append(offs[-1] + caps[s])

    with tile.TileContext(nc) as tc, ExitStack() as ctx:
        wpool = ctx.enter_context(tc.tile_pool(name="w", bufs=1))
        xpool = ctx.enter_context(tc.tile_pool(name="x", bufs=N_SPECIES))
        opool = ctx.enter_context(tc.tile_pool(name="o", bufs=2 * N_SPECIES))
        psA = ctx.enter_context(tc.tile_pool(name="psA", bufs=2, space="PSUM"))
        psB = ctx.enter_context(tc.tile_pool(name="psB", bufs=2, space="PSUM"))

        wt = wpool.tile([D_IN, N_SPECIES * N_OUT], MM_DT)
        # species-0 weights first so the first matmul can start early; the
        # rest of the weights stream behind it on the otherwise-idle ACT ring
        nc.scalar.dma_start(wt[:, :N_OUT], w[:, :N_OUT])
        nc.scalar.dma_start(wt[:, N_OUT:], w[:, N_OUT:])

        # input stream: one DMA per species segment on the SP ring, all
        # issued up-front (single-writer tiles -> no reuse stalls). The first
        # segment leads with a small piece so compute ramps immediately.
        xts = []
        for s in range(N_SPECIES):
            xt = xpool.tile([D_IN, max_cap], MM_DT, tag="x")
            xts.append(xt)
            cs = caps[s]
            pieces = [(0, CHUNK), (CHUNK, cs - CHUNK)] if s == 0 and cs > CHUNK \
                else [(0, cs)]
            for p0, pn in pieces:
                if pn > 0:
                    nc.sync.dma_start(xt[:, p0:p0 + pn],
                                      xT[:, offs[s] + p0:offs[s] + p0 + pn])

        # compute + drain. h-major per segment: all of h=0's matmuls share one
        # LDWEIGHTS (matmuls marked non-self-loading so consecutive matmuls
        # pipeline through the PE array instead of draining for a weight
        # reload), then h=1. The h=0 PSUM drain runs on DVE, h=1 on ACT, so
        # the two drains overlap across adjacent (s,h) phases. The remainder
        # column-group goes FIRST so the 3rd group's PSUM-buffer reuse only
        # waits on a tiny cast.
        for s in range(N_SPECIES):
            cs = caps[s]
            if cs == 0:
                continue
            ots = [opool.tile([128, max_cap], OUT_DT, tag="o", name=f"ot{s}h{h}")
                   for h in range(2)]
            rem = cs % GROUP
            groups = ([(0, rem)] if rem else []) + \
                     [(g0, GROUP) for g0 in range(rem, cs, GROUP)]
            for h in range(2):
                lhsT = wt[:, s * N_OUT + h * 128: s * N_OUT + h * 128 + 128]
                pool = psA if h == 0 else psB
                # output DMA piece boundaries: flush after the middle group and
                # after the last. Just-in-time half-size pieces keep the queue
                # backlog small (fair SDMA arbitration) and halve the final
                # drain piece. h=0 rides the SP ring (SP is idle after the
                # input issues), h=1 the ACT ring.
                deng = nc.sync if h == 0 else nc.scalar
                drow = outT[h * 128:(h + 1) * 128, offs[s]:offs[s] + cs]
                flushes = {len(groups) - 2, len(groups) - 1} if len(groups) > 1 \
                    else {0}
                q0 = 0
                for gi, (g0, gw) in enumerate(groups):
                    ps = pool.tile([128, GROUP], F32, tag="ps",
                                   name=f"ps{s}h{h}g{gi}")
                    for j in range(0, gw, CHUNK):
                        cj = min(CHUNK, gw - j)
                        nc.tensor.matmul(ps[:, j:j + cj], lhsT,
                                         xts[s][:, g0 + j:g0 + j + cj],
                                         start=True, stop=True)
                    if h == 0:
                        nc.vector.tensor_copy(ots[0][:, g0:g0 + gw], ps[:, :gw])
                    else:
                        nc.scalar.copy(ots[1][:, g0:g0 + gw], ps[:, :gw])
                    if gi in flushes:
                        deng.dma_start(drow[:, q0:g0 + gw],
                                       ots[h][:, q0:g0 + gw])
                        q0 = g0 + gw

    nc.compile()
    return nc


def _prepare(values, species_idx, combining_matrix):
    """Host routing + packing. Returns (in_maps, plan)."""
    values = np.ascontiguousarray(values, dtype=np.float32)
    species_idx = np.asarray(species_idx, dtype=np.int32)
    w_host = np.ascontiguousarray(
        np.asarray(combining_matrix, dtype=np.float32).transpose(1, 0, 2).reshape(
            D_IN, N_SPECIES * N_OUT).astype(BF16)
    )

    # per species, deal rows round-robin across cores (balanced +-1)
    core_rows = [[] for _ in range(N_CORES)]   # per core: list of row-index arrays
    counts = np.zeros((N_CORES, N_SPECIES), dtype=np.int64)
    for s in range(N_SPECIES):
        idx = np.nonzero(species_idx == s)[0]
        for c in range(N_CORES):
            sub = idx[c::N_CORES]
            core_rows[c].append(sub)
            counts[c, s] = sub.size

    caps = []
    for s in range(N_SPECIES):
        mx = int(counts[:, s].max())
        caps.append(0 if mx == 0 else -(-mx // PAD) * PAD)
    r_pad = int(sum(caps))
    offs = np.concatenate([[0], np.cumsum(caps)]).astype(np.int64)

    in_maps = []
    for c in range(N_CORES):
        xT = np.zeros((D_IN, r_pad), dtype=BF16)
        for s in range(N_SPECIES):
            n = counts[c, s]
            if n:
                xT[:, offs[s]:offs[s] + n] = values[core_rows[c][s]].astype(BF16).T
        in_maps.append({"xT": xT, "w": w_host})

    plan = {"core_rows": core_rows, "counts": counts, "caps": caps,
            "offs": offs, "r_pad": r_pad}
    return in_maps, plan


def _postprocess(results, plan):
    core_rows, counts, offs = plan["core_rows"], plan["counts"], plan["offs"]
    out = np.empty((M_TOTAL, N_OUT), dtype=np.float32)
    for c in range(N_CORES):
        oT = results[c]["outT"].astype(np.float32)
        for s in range(N_SPECIES):
            n = counts[c, s]
            if n:
                out[core_rows[c][s]] = oT[:, offs[s]:offs[s] + n].T
    return out


def kernel(values, species_idx, combining_matrix):
    in_maps, plan = _prepare(values, species_idx, combining_matrix)
    nc = _build_nc(plan["caps"], plan["r_pad"])
    res = run_bass_kernel_spmd(nc, in_maps, list(range(N_CORES)))
    return _postprocess(res.results, plan)


# revision 17
# speedup vs baseline: 1.0257x; 1.0257x over previous
"""Grouped-GEMM (MoE routing) kernel for TRN2, 8 NeuronCores, SPMD.

out[m] = values[m] @ combining_matrix[species_idx[m]]
  values [131072, 128] f32, species_idx [131072] i32, combining_matrix [8, 128, 256] f32

Strategy:
  - Host: counting-sort rows by species; deal each species' rows round-robin
    across the 8 cores so per-core per-species counts are balanced (+-1).
    Each core's rows are packed species-contiguous into a transposed buffer
    xT [128, R_pad] (species segment s zero-padded to a static capacity C[s],
    identical on every core -> one SPMD program). All device traffic is bf16
    (host casts f32<->bf16 for free): ~13 MB/core HBM traffic vs ~27 MB for
    f32; f32 PSUM accumulate keeps scale-relative error at ~3e-3 (gate 2e-2).
  - Device (per core): all 8 weight matrices resident in SBUF (one DMA).
    Per species s, output half h: out_T[h*128+.., seg_s] =
    W[s][:, h*128:+128].T @ xT[:, seg_s] via 512-col matmul chunks (bf16 in,
    f32 PSUM). The PSUM->SBUF drain (with bf16 downcast) is the throughput-
    critical non-DMA stage: PSUM-src copies run at 1x mode (~1 elem/cycle/
    lane), so the two halves are drained by DIFFERENT engines in parallel —
    h=0 on DVE (tensor_copy), h=1 on ACT (activation copy). Out-DMAs are
    issued by the engine that produced the data (SP ring for h=0, ACT ring
    for h=1) so each engine only ever waits on its own prior work.
  - Host: scatter outT columns back to the full [131072, 256] f32 output.
"""

import numpy as np
import ml_dtypes
from contextlib import ExitStack

import concourse.bass as bass
import concourse.mybir as mybir
import concourse.tile as tile
from concourse import bacc
from concourse.bass_utils import run_bass_kernel_spmd

M_TOTAL = 131072
D_IN = 128
N_OUT = 256
N_SPECIES = 8
N_CORES = 8
PAD = 16           # species segment capacity granularity (rows)
CHUNK = 512        # matmul moving-dim chunk (PSUM bank limit)
GROUP = 1024       # PSUM tile cols (2 banks) drained by one cast
F32 = mybir.dt.float32
BF16 = ml_dtypes.bfloat16
MM_DT = mybir.dt.bfloat16
OUT_DT = mybir.dt.bfloat16


def _build_nc(caps, r_pad):
    """Build the SPMD program for one core. caps[s] = padded column count of
    species segment s (same on all cores); r_pad = sum(caps)."""
    nc = bacc.Bacc("TRN2", target_bir_lowering=False, debug=False,
                   num_devices=N_CORES)
    xT = nc.dram_tensor("xT", [D_IN, r_pad], MM_DT, kind="ExternalInput").ap()
    w = nc.dram_tensor("w", [D_IN, N_SPECIES * N_OUT], MM_DT,
                       kind="ExternalInput").ap()
    outT = nc.dram_tensor("outT", [N_OUT, r_pad], OUT_DT, kind="ExternalOutput").ap()

    max_cap = max(caps)
    offs = [0]
    for s in range(N_SPECIES):
        offs.append(offs[-1] + caps[s])

    with tile.TileContext(nc) as tc, ExitStack() as ctx:
        wpool = ctx.enter_context(tc.tile_pool(name="w", bufs=1))
        xpool = ctx.enter_context(tc.tile_pool(name="x", bufs=N_SPECIES))
        opool = ctx.enter_context(tc.tile_pool(name="o", bufs=2 * N_SPECIES))
        psA = ctx.enter_context(tc.tile_pool(name="psA", bufs=2, space="PSUM"))
        psB = ctx.enter_context(tc.tile_pool(name="psB", bufs=2, space="PSUM"))

        wt = wpool.tile([D_IN, N_SPECIES * N_OUT], MM_DT)
        # species-0 weights first so the first matmul can start early; the
        # rest of the weights stream behind it on the otherwise-idle ACT ring
        nc.scalar.dma_start(wt[:, :N_OUT], w[:, :N_OUT])
        nc.scalar.dma_start(wt[:, N_OUT:], w[:, N_OUT:])

        # input stream: one DMA per species segment on the SP ring, all
        # issued up-front (single-writer tiles -> no reuse stalls). The first
        # segment leads with a small piece so compute ramps immediately.
        xts = []
        for s in range(N_SPECIES):
            xt = xpool.tile([D_IN, max_cap], MM_DT, tag="x")
            xts.append(xt)
            cs = caps[s]
            pieces = [(0, CHUNK), (CHUNK, cs - CHUNK)] if s == 0 and cs > CHUNK \
                else [(0, cs)]
            for p0, pn in pieces:
                if pn > 0:
                    nc.sync.dma_start(xt[:, p0:p0 + pn],
                                      xT[:, offs[s] + p0:offs[s] + p0 + pn])

        # compute + drain, h-major per segment. The h=0 PSUM drain runs on
        # DVE, h=1 on ACT, so the two drains overlap across adjacent (s,h)
        # phases. The remainder column-group goes FIRST so the 3rd group's
        # PSUM-buffer reuse only waits on a tiny cast.
        for s in range(N_SPECIES):
            cs = caps[s]
            if cs == 0:
                continue
            ots = [opool.tile([128, max_cap], OUT_DT, tag="o", name=f"ot{s}h{h}")
                   for h in range(2)]
            rem = cs % GROUP
            groups = ([(0, rem)] if rem else []) + \
                     [(g0, GROUP) for g0 in range(rem, cs, GROUP)]
            for h in range(2):
                lhsT = wt[:, s * N_OUT + h * 128: s * N_OUT + h * 128 + 128]
                pool = psA if h == 0 else psB
                # one output DMA per half-segment, issued after its last cast
                # (finer per-group pieces measured slower: the ~600ns per-issue
                # cost on SP/ACT outweighs the smoother queue backlog). h=0
                # rides the SP ring (SP is idle after the input issues), h=1
                # the ACT ring so the two output streams overlap.
                deng = nc.sync if h == 0 else nc.scalar
                drow = outT[h * 128:(h + 1) * 128, offs[s]:offs[s] + cs]
                flushes = {len(groups) - 1}
                q0 = 0
                for gi, (g0, gw) in enumerate(groups):
                    ps = pool.tile([128, GROUP], F32, tag="ps",
                                   name=f"ps{s}h{h}g{gi}")
                    for j in range(0, gw, CHUNK):
                        cj = min(CHUNK, gw - j)
                        nc.tensor.matmul(ps[:, j:j + cj], lhsT,
                                         xts[s][:, g0 + j:g0 + j + cj],
                                         start=True, stop=True)
                    if h == 0:
                        nc.vector.tensor_copy(ots[0][:, g0:g0 + gw], ps[:, :gw])
                    else:
                        nc.scalar.copy(ots[1][:, g0:g0 + gw], ps[:, :gw])
                    if gi in flushes:
                        deng.dma_start(drow[:, q0:g0 + gw],
                                       ots[h][:, q0:g0 + gw])
                        q0 = g0 + gw

    nc.compile()
    return nc


def _prepare(values, species_idx, combining_matrix):
    """Host routing + packing. Returns (in_maps, plan)."""
    values = np.ascontiguousarray(values, dtype=np.float32)
    species_idx = np.asarray(species_idx, dtype=np.int32)
    w_host = np.ascontiguousarray(
        np.asarray(combining_matrix, dtype=np.float32).transpose(1, 0, 2).reshape(
            D_IN, N_SPECIES * N_OUT).astype(BF16)
    )

    # per species, deal rows round-robin across cores (balanced +-1)
    core_rows = [[] for _ in range(N_CORES)]   # per core: list of row-index arrays
    counts = np.zeros((N_CORES, N_SPECIES), dtype=np.int64)
    for s in range(N_SPECIES):
        idx = np.nonzero(species_idx == s)[0]
        for c in range(N_CORES):
            sub = idx[c::N_CORES]
            core_rows[c].append(sub)
            counts[c, s] = sub.size

    caps = []
    for s in range(N_SPECIES):
        mx = int(counts[:, s].max())
        caps.append(0 if mx == 0 else -(-mx // PAD) * PAD)
    r_pad = int(sum(caps))
    offs = np.concatenate([[0], np.cumsum(caps)]).astype(np.int64)

    in_maps = []
    for c in range(N_CORES):
        xT = np.zeros((D_IN, r_pad), dtype=BF16)
        for s in range(N_SPECIES):
            n = counts[c, s]
            if n:
                xT[:, offs[s]:offs[s] + n] = values[core_rows[c][s]].astype(BF16).T
        in_maps.append({"xT": xT, "w": w_host})

    plan = {"core_rows": core_rows, "counts": counts, "caps": caps,
            "offs": offs, "r_pad": r_pad}
    return in_maps, plan


def _postprocess(results, plan):
    core_rows, counts, offs = plan["core_rows"], plan["counts"], plan["offs"]
    out = np.empty((M_TOTAL, N_OUT), dtype=np.float32)
    for c in range(N_CORES):
        oT = results[c]["outT"].astype(np.float32)
        for s in range(N_SPECIES):
            n = counts[c, s]
            if n:
                out[core_rows[c][s]] = oT[:, offs[s]:offs[s] + n].T
    return out


def kernel(values, species_idx, combining_matrix):
    in_maps, plan = _prepare(values, species_idx, combining_matrix)
    nc = _build_nc(plan["caps"], plan["r_pad"])
    res = run_bass_kernel_spmd(nc, in_maps, list(range(N_CORES)))
    return _postprocess(res.results, plan)


# revision 18
# speedup vs baseline: 1.0345x; 1.0086x over previous
"""Grouped-GEMM (MoE routing) kernel for TRN2, 8 NeuronCores, SPMD.

out[m] = values[m] @ combining_matrix[species_idx[m]]
  values [131072, 128] f32, species_idx [131072] i32, combining_matrix [8, 128, 256] f32

Strategy:
  - Host: counting-sort rows by species; deal each species' rows round-robin
    across the 8 cores so per-core per-species counts are balanced (+-1).
    Each core's rows are packed species-contiguous into a transposed buffer
    xT [128, R_pad] (species segment s zero-padded to a static capacity C[s],
    identical on every core -> one SPMD program). All device traffic is bf16
    (host casts f32<->bf16 for free): ~13 MB/core HBM traffic vs ~27 MB for
    f32; f32 PSUM accumulate keeps scale-relative error at ~3e-3 (gate 2e-2).
  - Device (per core): all 8 weight matrices resident in SBUF (one DMA).
    Per species s, output half h: out_T[h*128+.., seg_s] =
    W[s][:, h*128:+128].T @ xT[:, seg_s] via 512-col matmul chunks (bf16 in,
    f32 PSUM). The PSUM->SBUF drain (with bf16 downcast) is the throughput-
    critical non-DMA stage: PSUM-src copies run at 1x mode (~1 elem/cycle/
    lane), so the two halves are drained by DIFFERENT engines in parallel —
    h=0 on DVE (tensor_copy), h=1 on ACT (activation copy). Out-DMAs are
    issued by the engine that produced the data (SP ring for h=0, ACT ring
    for h=1) so each engine only ever waits on its own prior work.
  - Host: scatter outT columns back to the full [131072, 256] f32 output.
"""

import numpy as np
import ml_dtypes
from contextlib import ExitStack

import concourse.bass as bass
import concourse.mybir as mybir
import concourse.tile as tile
from concourse import bacc
from concourse.bass_utils import run_bass_kernel_spmd

M_TOTAL = 131072
D_IN = 128
N_OUT = 256
N_SPECIES = 8
N_CORES = 8
PAD = 16           # species segment capacity granularity (rows)
CHUNK = 512        # matmul moving-dim chunk (PSUM bank limit)
GROUP = 1024       # PSUM tile cols (2 banks) drained by one cast
F32 = mybir.dt.float32
BF16 = ml_dtypes.bfloat16
MM_DT = mybir.dt.bfloat16
OUT_DT = mybir.dt.bfloat16


def _build_nc(caps, r_pad):
    """Build the SPMD program for one core. caps[s] = padded column count of
    species segment s (same on all cores); r_pad = sum(caps)."""
    nc = bacc.Bacc("TRN2", target_bir_lowering=False, debug=False,
                   num_devices=N_CORES)
    xT = nc.dram_tensor("xT", [D_IN, r_pad], MM_DT, kind="ExternalInput").ap()
    w = nc.dram_tensor("w", [D_IN, N_SPECIES * N_OUT], MM_DT,
                       kind="ExternalInput").ap()
    outT = nc.dram_tensor("outT", [N_OUT, r_pad], OUT_DT, kind="ExternalOutput").ap()

    max_cap = max(caps)
    offs = [0]
    for s in range(N_SPECIES):
        offs.append(offs[-1] + caps[s])

    with tile.TileContext(nc) as tc, ExitStack() as ctx:
        wpool = ctx.enter_context(tc.tile_pool(name="w", bufs=1))
        xpool = ctx.enter_context(tc.tile_pool(name="x", bufs=N_SPECIES))
        opool = ctx.enter_context(tc.tile_pool(name="o", bufs=2 * N_SPECIES))
        psA = ctx.enter_context(tc.tile_pool(name="psA", bufs=2, space="PSUM"))
        psB = ctx.enter_context(tc.tile_pool(name="psB", bufs=2, space="PSUM"))

        wt = wpool.tile([D_IN, N_SPECIES * N_OUT], MM_DT)
        # species-0 weights first so the first matmul can start early; the
        # rest of the weights stream behind it on the otherwise-idle ACT ring
        nc.scalar.dma_start(wt[:, :N_OUT], w[:, :N_OUT])
        nc.scalar.dma_start(wt[:, N_OUT:], w[:, N_OUT:])

        # input stream: one DMA per species segment on the SP ring, all
        # issued up-front (single-writer tiles -> no reuse stalls). The first
        # segment leads with a small piece so compute ramps immediately.
        xts = []
        for s in range(N_SPECIES):
            xt = xpool.tile([D_IN, max_cap], MM_DT, tag="x")
            xts.append(xt)
            cs = caps[s]
            pieces = [(0, CHUNK), (CHUNK, cs - CHUNK)] if s == 0 and cs > CHUNK \
                else [(0, cs)]
            for p0, pn in pieces:
                if pn > 0:
                    nc.sync.dma_start(xt[:, p0:p0 + pn],
                                      xT[:, offs[s] + p0:offs[s] + p0 + pn])

        # compute + drain, h-major per segment. The h=0 PSUM drain runs on
        # DVE, h=1 on ACT, so the two drains overlap across adjacent (s,h)
        # phases. The remainder column-group goes FIRST so the 3rd group's
        # PSUM-buffer reuse only waits on a tiny cast.
        for s in range(N_SPECIES):
            cs = caps[s]
            if cs == 0:
                continue
            ots = [opool.tile([128, max_cap], OUT_DT, tag="o", name=f"ot{s}h{h}")
                   for h in range(2)]
            rem = cs % GROUP
            groups = ([(0, rem)] if rem else []) + \
                     [(g0, GROUP) for g0 in range(rem, cs, GROUP)]
            for h in range(2):
                lhsT = wt[:, s * N_OUT + h * 128: s * N_OUT + h * 128 + 128]
                pool = psA if h == 0 else psB
                # one output DMA per half-segment, issued after its last cast
                # (finer per-group pieces measured slower: the ~600ns per-issue
                # cost on SP/ACT outweighs the smoother queue backlog). h=0
                # rides the SP ring (SP is idle after the input issues), h=1
                # the ACT ring so the two output streams overlap.
                deng = nc.sync if h == 0 else nc.scalar
                drow = outT[h * 128:(h + 1) * 128, offs[s]:offs[s] + cs]
                # the LAST segment flushes in two pieces so the final drain
                # piece (after the very last cast) is half-size; elsewhere one
                # piece per half-segment minimizes the ~600ns per-issue cost
                if s == N_SPECIES - 1 and len(groups) > 1:
                    flushes = {len(groups) - 2, len(groups) - 1}
                else:
                    flushes = {len(groups) - 1}
                q0 = 0
                for gi, (g0, gw) in enumerate(groups):
                    ps = pool.tile([128, GROUP], F32, tag="ps",
                                   name=f"ps{s}h{h}g{gi}")
                    for j in range(0, gw, CHUNK):
                        cj = min(CHUNK, gw - j)
                        nc.tensor.matmul(ps[:, j:j + cj], lhsT,
                                         xts[s][:, g0 + j:g0 + j + cj],
                                         start=True, stop=True)
                    if h == 0:
                        nc.vector.tensor_copy(ots[0][:, g0:g0 + gw], ps[:, :gw])
                    else:
                        nc.scalar.copy(ots[1][:, g0:g0 + gw], ps[:, :gw])
                    if gi in flushes:
                        deng.dma_start(drow[:, q0:g0 + gw],
                                       ots[h][:, q0:g0 + gw])
                        q0 = g0 + gw

    nc.compile()
    return nc


def _prepare(values, species_idx, combining_matrix):
    """Host routing + packing. Returns (in_maps, plan)."""
    values = np.ascontiguousarray(values, dtype=np.float32)
    species_idx = np.asarray(species_idx, dtype=np.int32)
    w_host = np.ascontiguousarray(
        np.asarray(combining_matrix, dtype=np.float32).transpose(1, 0, 2).reshape(
            D_IN, N_SPECIES * N_OUT).astype(BF16)
    )

    # per species, deal rows round-robin across cores (balanced +-1)
    core_rows = [[] for _ in range(N_CORES)]   # per core: list of row-index arrays
    counts = np.zeros((N_CORES, N_SPECIES), dtype=np.int64)
    for s in range(N_SPECIES):
        idx = np.nonzero(species_idx == s)[0]
        for c in range(N_CORES):
            sub = idx[c::N_CORES]
            core_rows[c].append(sub)
            counts[c, s] = sub.size

    caps = []
    for s in range(N_SPECIES):
        mx = int(counts[:, s].max())
        caps.append(0 if mx == 0 else -(-mx // PAD) * PAD)
    r_pad = int(sum(caps))
    offs = np.concatenate([[0], np.cumsum(caps)]).astype(np.int64)

    in_maps = []
    for c in range(N_CORES):
        xT = np.zeros((D_IN, r_pad), dtype=BF16)
        for s in range(N_SPECIES):
            n = counts[c, s]
            if n:
                xT[:, offs[s]:offs[s] + n] = values[core_rows[c][s]].astype(BF16).T
        in_maps.append({"xT": xT, "w": w_host})

    plan = {"core_rows": core_rows, "counts": counts, "caps": caps,
            "offs": offs, "r_pad": r_pad}
    return in_maps, plan


def _postprocess(results, plan):
    core_rows, counts, offs = plan["core_rows"], plan["counts"], plan["offs"]
    out = np.empty((M_TOTAL, N_OUT), dtype=np.float32)
    for c in range(N_CORES):
        oT = results[c]["outT"].astype(np.float32)
        for s in range(N_SPECIES):
            n = counts[c, s]
            if n:
                out[core_rows[c][s]] = oT[:, offs[s]:offs[s] + n].T
    return out


def kernel(values, species_idx, combining_matrix):
    in_maps, plan = _prepare(values, species_idx, combining_matrix)
    nc = _build_nc(plan["caps"], plan["r_pad"])
    res = run_bass_kernel_spmd(nc, in_maps, list(range(N_CORES)))
    return _postprocess(res.results, plan)
